# revision 1
# baseline (speedup 1.0000x reference)
"""GNN message-passing layer (out = relu(segment_sum(vals * (xW)[src] by dst)))
on 8 Trainium2 NeuronCores.

Strategy (1D graph partitioning, per sharding hint):
- dst nodes are permuted into 8*BLOCKS blocks of 128, degree-balanced so
  every block has <= C*128 incoming edges; core m owns blocks
  [m*BLOCKS, (m+1)*BLOCKS) and produces those output rows.
- Every core computes the full dense transform h = x @ W (replicated; avoids
  any cross-core communication) into its DRAM, via float32r matmuls.
- Per dst block: one indirect DMA gathers the C*128 source rows h[src] into
  SBUF; a value-scaled one-hot matrix P (built on DVE from iota/is_equal) is
  matmul'd against the messages, accumulating all chunks into one PSUM tile:
  psum[d, f] = sum_e val[e] * [dst_e == d] * h[src_e, f]; relu + store.
- Padding edges carry val = 0 so they contribute nothing.
"""
import math
from contextlib import ExitStack

import numpy as np

import concourse.bass as bass
import concourse.mybir as mybir
import concourse.tile as tile
from concourse.bass_utils import run_bass_kernel_spmd
from concourse.vector_clock import ScopedClock

# --- workaround: this walrus build rejects >1 sync wait per instruction
# ("Too many sync wait commands"). Tile's kernel-tail drain carries one wait
# per active sem lane; give it the same NOP-splitting treatment as everything
# else via a post-schedule legalization pass over all basic blocks. ---
_MAX_WAITS = 1


def _patched_drain_and_barrier(self, tick_clock, wait_clock):
    drain_inst = self.nc.sync.drain()
    wait_clock.add_sem_waits(
        drain_inst.ins, ScopedClock({None: tick_clock.global_clock})
    )
    self.nc.all_engine_barrier()
    popped = self.nc._tile_sem_poison_stack.pop()
    assert popped is self._sem_poison
    self.nc.clear_and_free_semaphores(list(self.sems.allocated().values()))
    self.nc.all_engine_barrier()


tile.TileContext._drain_and_barrier = _patched_drain_and_barrier


def _legalize_sync_waits(nc):
    """Split instructions carrying >_MAX_WAITS sem waits: excess waits move to
    same-engine NOPs inserted immediately before the instruction."""
    n_split = 0
    for f in nc.m.functions:
        for bb in f.blocks:
            out = []
            changed = False
            for ins in bb.instructions:
                si = ins.sync_info
                waits = list(si.on_wait) if si and si.on_wait else []
                if len(waits) > _MAX_WAITS:
                    changed = True
                    n_split += 1
                    for i in range(_MAX_WAITS, len(waits), _MAX_WAITS):
                        nop = mybir.InstNoOp(
                            name=nc.get_next_instruction_name(), ins=[], outs=[]
                        )
                        nop.engine = ins.engine
                        nop.sync_info = mybir.SyncInfo(
                            on_wait=waits[i : i + _MAX_WAITS], on_update=[]
                        )
                        nc.register_instruction(nop, overwrite=True)
                        out.append(nop)
                    si.on_wait = waits[:_MAX_WAITS]
                out.append(ins)
            if changed:
                bb.instructions = out
    return n_split

N_CORES = 8
P = 128


def build_nc(n_pad, d_in, d_out, blocks, C, strip_blocks=8, phase_barrier=False, debug_h=False):
    """One SPMD program. n_pad: padded node count (h rows, mult of 128).
    blocks: dst blocks per core. C: chunks (of 128 edges) per block."""
    f32 = mybir.dt.float32
    f32r = mybir.dt.float32r
    bf16 = mybir.dt.bfloat16
    i32 = mybir.dt.int32
    KD = d_in // P
    NB = n_pad // P

    nc = bass.Bass()
    xT = nc.declare_dram_parameter("xT", [d_in, n_pad], f32r, isOutput=False)
    Wp = nc.declare_dram_parameter("W", [d_in, d_out], f32r, isOutput=False)
    srcp = nc.declare_dram_parameter("src", [P, blocks * C], i32, isOutput=False)
    dstp = nc.declare_dram_parameter("dstv", [P, blocks * C], f32, isOutput=False)
    valp = nc.declare_dram_parameter("valv", [P, blocks * C], f32, isOutput=False)
    outp = nc.declare_dram_parameter("out", [blocks * P, d_out], f32, isOutput=True)
    h = nc.dram_tensor("h", [n_pad, d_out], bf16)
    hout = (
        nc.declare_dram_parameter("hout", [n_pad, d_out], bf16, isOutput=True)
        if debug_h
        else None
    )

    with tile.TileContext(nc) as tc:
        with ExitStack() as ctx:
            wpool = ctx.enter_context(tc.tile_pool(name="w", bufs=1))
            epool = ctx.enter_context(tc.tile_pool(name="edges", bufs=1))
            xpool = ctx.enter_context(tc.tile_pool(name="xs", bufs=2))
            hpool = ctx.enter_context(tc.tile_pool(name="hs", bufs=2))
            ps1 = ctx.enter_context(tc.tile_pool(name="ps1", bufs=4, space="PSUM"))
            mpool = ctx.enter_context(tc.tile_pool(name="msgs", bufs=2))
            ppool = ctx.enter_context(tc.tile_pool(name="onehot", bufs=2))
            ps2 = ctx.enter_context(tc.tile_pool(name="ps2", bufs=4, space="PSUM"))
            opool = ctx.enter_context(tc.tile_pool(name="osb", bufs=2))

            # --- constants / per-core edge data, loaded once ---
            w_t = wpool.tile([P, KD * d_out], f32r)
            for k in range(KD):
                nc.sync.dma_start(
                    w_t[:, k * d_out : (k + 1) * d_out], Wp[k * P : (k + 1) * P, :]
                )
            src_t = epool.tile([P, blocks * C], i32)
            dst_t = epool.tile([P, blocks * C], f32)
            val_t = epool.tile([P, blocks * C], f32)
            nc.sync.dma_start(src_t[:], srcp[:])
            nc.sync.dma_start(dst_t[:], dstp[:])
            nc.sync.dma_start(val_t[:], valp[:])
            iota_t = epool.tile([P, C * P], f32)
            nc.gpsimd.iota(
                iota_t[:],
                pattern=[[0, C], [1, P]],
                base=0,
                channel_multiplier=0,
                allow_small_or_imprecise_dtypes=True,
            )

            # --- phase 1: h = x @ W (all n_pad rows, replicated) ---
            SBN = strip_blocks * P  # nodes per strip
            for s0 in range(0, NB, strip_blocks):
                nbc = min(strip_blocks, NB - s0)
                sn = nbc * P
                xs = xpool.tile([P, KD * SBN], f32r, tag="xs")
                for k in range(KD):
                    nc.sync.dma_start(
                        xs[:, k * SBN : k * SBN + sn],
                        xT[k * P : (k + 1) * P, s0 * P : s0 * P + sn],
                    )
                hs = hpool.tile([P, strip_blocks * d_out], bf16, tag="hs")
                for j in range(nbc):
                    pt = ps1.tile([P, d_out], f32)
                    for k in range(KD):
                        nc.tensor.matmul(
                            pt[:],
                            lhsT=xs[:, k * SBN + j * P : k * SBN + (j + 1) * P],
                            rhs=w_t[:, k * d_out : (k + 1) * d_out],
                            start=(k == 0),
                            stop=(k == KD - 1),
                        )
                    nc.vector.tensor_copy(hs[:, j * d_out : (j + 1) * d_out], pt[:])
                hd = h[s0 * P : s0 * P + sn, :].rearrange("(nb p) f -> p nb f", p=P)
                hsv = hs[:, : nbc * d_out].rearrange("p (nb f) -> p nb f", nb=nbc)
                nc.sync.dma_start(hd, hsv)
                if debug_h:
                    hodv = hout[s0 * P : s0 * P + sn, :].rearrange(
                        "(nb p) f -> p nb f", p=P
                    )
                    nc.sync.dma_start(hodv, hsv)

            if phase_barrier:
                tc.strict_bb_all_engine_barrier()
            # --- phase 2: gather + one-hot scatter matmul per dst block ---
            for b in range(blocks):
                msgs = mpool.tile([P, C * d_out], bf16, tag="msgs")
                # HW indirect DMA honors one offset per partition per
                # instruction (the sim's multi-column walk does not exist on
                # HW) -> one gather of 128 rows per chunk.
                for c in range(C):
                    nc.gpsimd.indirect_dma_start(
                        out=msgs[:, c * d_out : (c + 1) * d_out],
                        out_offset=None,
                        in_=h[:],
                        in_offset=bass.IndirectOffsetOnAxis(
                            ap=src_t[:, b * C + c : b * C + c + 1], axis=0
                        ),
                    )
                pt3 = ppool.tile([P, C * P], bf16, tag="P")
                iota3 = bass.AP(
                    iota_t[:].tensor, iota_t[:].offset,
                    [iota_t[:].ap[0], [P, C], [1, P]],
                )
                p3 = bass.AP(
                    pt3[:].tensor, pt3[:].offset, [pt3[:].ap[0], [P, C], [1, P]]
                )
                dstb = dst_t[:, b * C : (b + 1) * C]
                valb = val_t[:, b * C : (b + 1) * C]
                dst_bc = bass.AP(dstb.tensor, dstb.offset, dstb.ap + [[0, P]])
                val_bc = bass.AP(valb.tensor, valb.offset, valb.ap + [[0, P]])
                nc.vector.tensor_tensor(
                    out=p3, in0=iota3, in1=dst_bc, op=mybir.AluOpType.is_equal
                )
                nc.vector.tensor_tensor(
                    out=p3, in0=p3, in1=val_bc, op=mybir.AluOpType.mult
                )
                acc = ps2.tile([P, d_out], f32)
                for c in range(C):
                    nc.tensor.matmul(
                        acc[:],
                        lhsT=pt3[:, c * P : (c + 1) * P],
                        rhs=msgs[:, c * d_out : (c + 1) * d_out],
                        start=(c == 0),
                        stop=(c == C - 1),
                    )
                ot = opool.tile([P, d_out], f32)
                nc.scalar.activation(ot[:], acc[:], mybir.ActivationFunctionType.Relu)
                nc.sync.dma_start(outp[b * P : (b + 1) * P, :], ot[:])
    _legalize_sync_waits(nc)
    return nc


def _pack_edges(edge_src, edge_dst, edge_vals, n_nodes, blocks):
    """Permute dst nodes into degree-balanced blocks of 128; pack edges into
    [P, blocks_total*C] per-core arrays (chunk-major columns per block)."""
    import heapq

    total_blocks = N_CORES * blocks
    deg = np.bincount(edge_dst, minlength=n_nodes).astype(np.int64)
    order = np.argsort(-deg, kind="stable")
    heap = [(0, b) for b in range(total_blocks)]
    heapq.heapify(heap)
    counts = np.zeros(total_blocks, np.int32)
    loads = np.zeros(total_blocks, np.int64)
    block_of = np.empty(n_nodes, np.int32)
    slot_of = np.empty(n_nodes, np.int32)
    for node in order:
        while True:
            load, b = heapq.heappop(heap)
            if counts[b] < P:
                break
        block_of[node] = b
        slot_of[node] = counts[b]
        counts[b] += 1
        loads[b] = load + deg[node]
        if counts[b] < P:
            heapq.heappush(heap, (loads[b], b))
    C = max(1, math.ceil(loads.max() / P))

    eb = block_of[edge_dst]
    eorder = np.argsort(eb, kind="stable")
    eb_sorted = eb[eorder]
    bsizes = np.bincount(eb_sorted, minlength=total_blocks)
    boffs = np.concatenate([[0], np.cumsum(bsizes)[:-1]])
    pos = np.arange(len(edge_src)) - boffs[eb_sorted]

    src_a = np.zeros((total_blocks, C, P), np.int32)
    dst_a = np.zeros((total_blocks, C, P), np.float32)
    val_a = np.zeros((total_blocks, C, P), np.float32)
    ch, lane = pos >> 7, pos & 127
    src_a[eb_sorted, ch, lane] = edge_src[eorder]
    dst_a[eb_sorted, ch, lane] = slot_of[edge_dst[eorder]]
    val_a[eb_sorted, ch, lane] = edge_vals[eorder]

    def per_core(a):
        return [
            np.ascontiguousarray(
                a[m * blocks : (m + 1) * blocks].transpose(2, 0, 1).reshape(P, -1)
            )
            for m in range(N_CORES)
        ]

    return per_core(src_a), per_core(dst_a), per_core(val_a), block_of, slot_of, C


def _run(x, W, edge_vals, edge_src, edge_dst, blocks=None, trace=False, phase_barrier=False):
    n_nodes, d_in = x.shape
    d_out = W.shape[1]
    if blocks is None:
        blocks = math.ceil(n_nodes / (N_CORES * P))
    n_pad = math.ceil(n_nodes / P) * P
    src_c, dst_c, val_c, block_of, slot_of, C = _pack_edges(
        edge_src, edge_dst, edge_vals, n_nodes, blocks
    )
    xp = np.zeros((n_pad, d_in), np.float32)
    xp[:n_nodes] = x
    xT = np.ascontiguousarray(xp.T)

    nc = build_nc(n_pad, d_in, d_out, blocks, C, phase_barrier=phase_barrier)
    in_maps = [
        {"xT": xT, "W": W, "src": src_c[m], "dstv": dst_c[m], "valv": val_c[m]}
        for m in range(N_CORES)
    ]
    res = run_bass_kernel_spmd(nc, in_maps, list(range(N_CORES)), trace=trace)
    stacked = np.concatenate([res.results[m]["out"] for m in range(N_CORES)], axis=0)
    npc = blocks * P
    gidx = (block_of // blocks) * npc + (block_of % blocks) * P + slot_of
    out = stacked[gidx]
    return out, res


def kernel(x, W, edge_vals, edge_src, edge_dst):
    x = np.asarray(x, np.float32)
    W = np.asarray(W, np.float32)
    edge_vals = np.asarray(edge_vals, np.float32)
    edge_src = np.asarray(edge_src).astype(np.int64)
    edge_dst = np.asarray(edge_dst).astype(np.int64)
    out, _ = _run(x, W, edge_vals, edge_src, edge_dst)
    return out.astype(np.float32)



# revision 6
# speedup vs baseline: 1.8563x; 1.8563x over previous
"""GNN message-passing layer (out = relu(segment_sum(vals * (xW)[src] by dst)))
on 8 Trainium2 NeuronCores.

Strategy (1D graph partitioning, per sharding hint):
- dst nodes are permuted into 8*BLOCKS blocks of 128, degree-balanced so
  every block has bounded incoming edges; core m owns blocks
  [m*BLOCKS, (m+1)*BLOCKS) and produces those output rows.
- Every core computes the full dense transform h = x @ W in bf16 (replicated;
  avoids cross-core communication), written as two DRAM tensors h_lo/h_hi
  (node halves) so phase 2's low-half gathers can start while the high half
  is still being computed.
- Per dst block and src half: one batched SWDGE dma_gather pulls all C*128
  source rows h[src] into SBUF in [lane, chunk, feat] layout (idx i ->
  partition i%128, chunk i//128), amortizing the ~1us fixed SWDGE cost over
  the whole block instead of paying it per 128-row chunk. int16 gather
  indices only span 32K rows, hence the lo/hi split.
- A value-scaled one-hot matrix P (DVE iota/is_equal, bf16) is matmul'd
  against the messages, accumulating into PSUM:
  psum[d, f] = sum_e val[e] * [dst_e == d] * h[src_e, f]; low-half partial
  sums park in SBUF f32; high-half pass adds, relu, store.
- Padding edges carry val = 0 and gather row 0, so they contribute nothing.
"""
import math
from contextlib import ExitStack

import numpy as np
import ml_dtypes

import concourse.bass as bass
import concourse.mybir as mybir
import concourse.tile as tile
from concourse.bass_utils import run_bass_kernel_spmd
from concourse.library_config import mlp
from concourse.library_overlay import lower_extended_insts
from concourse.vector_clock import ScopedClock

# --- workaround: this walrus build rejects >1 sync wait per instruction
# ("Too many sync wait commands"). Tile's kernel-tail drain carries one wait
# per active sem lane; give it the same NOP-splitting treatment as everything
# else via a post-schedule legalization pass over all basic blocks. ---
_MAX_WAITS = 1


def _patched_drain_and_barrier(self, tick_clock, wait_clock):
    drain_inst = self.nc.sync.drain()
    wait_clock.add_sem_waits(
        drain_inst.ins, ScopedClock({None: tick_clock.global_clock})
    )
    self.nc.all_engine_barrier()
    popped = self.nc._tile_sem_poison_stack.pop()
    assert popped is self._sem_poison
    self.nc.clear_and_free_semaphores(list(self.sems.allocated().values()))
    self.nc.all_engine_barrier()


tile.TileContext._drain_and_barrier = _patched_drain_and_barrier


def _legalize_sync_waits(nc):
    """Split instructions carrying >_MAX_WAITS sem waits: excess waits move to
    same-engine NOPs inserted immediately before the instruction."""
    n_split = 0
    for f in nc.m.functions:
        for bb in f.blocks:
            out = []
            changed = False
            for ins in bb.instructions:
                si = ins.sync_info
                waits = list(si.on_wait) if si and si.on_wait else []
                if len(waits) > _MAX_WAITS:
                    changed = True
                    n_split += 1
                    for i in range(_MAX_WAITS, len(waits), _MAX_WAITS):
                        nop = mybir.InstNoOp(
                            name=nc.get_next_instruction_name(), ins=[], outs=[]
                        )
                        nop.engine = ins.engine
                        nop.sync_info = mybir.SyncInfo(
                            on_wait=waits[i : i + _MAX_WAITS], on_update=[]
                        )
                        nc.register_instruction(nop, overwrite=True)
                        out.append(nop)
                    si.on_wait = waits[:_MAX_WAITS]
                out.append(ins)
            if changed:
                bb.instructions = out
    return n_split

N_CORES = 8
P = 128


def build_nc(n_pad, H, d_in, d_out, blocks, C_lo, C_hi, strip_blocks=8):
    """One SPMD program. n_pad: padded node count (mult of 128), H: low-half
    row count (mult of 128, < 32768). blocks: dst blocks per core. C_lo/C_hi:
    chunks (of 128 edges) per block for the low/high src halves."""
    f32 = mybir.dt.float32
    bf16 = mybir.dt.bfloat16
    i16 = mybir.dt.int16
    KD = d_in // P
    C = C_lo + C_hi
    Cmax = max(C_lo, C_hi)
    NB_lo = H // P
    NB_hi = (n_pad - H) // P

    nc = bass.Bass(num_swdge_queues=4)
    xT = nc.declare_dram_parameter("xT", [d_in, n_pad], bf16, isOutput=False)
    Wp = nc.declare_dram_parameter("W", [d_in, d_out], bf16, isOutput=False)
    idxp = nc.declare_dram_parameter("idx", [P, blocks * C * 8], i16, isOutput=False)
    dstp = nc.declare_dram_parameter("dstv", [P, blocks * C], bf16, isOutput=False)
    valp = nc.declare_dram_parameter("valv", [P, blocks * C], bf16, isOutput=False)
    outp = nc.declare_dram_parameter("out", [blocks * P, d_out], f32, isOutput=True)
    h_lo = nc.dram_tensor("h_lo", [H, d_out], bf16)
    h_hi = nc.dram_tensor("h_hi", [n_pad - H, d_out], bf16)

    with tile.TileContext(nc) as tc:
        with ExitStack() as ctx:
            wpool = ctx.enter_context(tc.tile_pool(name="w", bufs=1))
            epool = ctx.enter_context(tc.tile_pool(name="edges", bufs=1))
            xpool = ctx.enter_context(tc.tile_pool(name="xs", bufs=2))
            hpool = ctx.enter_context(tc.tile_pool(name="hs", bufs=2))
            ps1 = ctx.enter_context(tc.tile_pool(name="ps1", bufs=4, space="PSUM"))
            mpool = ctx.enter_context(tc.tile_pool(name="msgs", bufs=3))
            ppool = ctx.enter_context(tc.tile_pool(name="onehot", bufs=3))
            ps2 = ctx.enter_context(tc.tile_pool(name="ps2", bufs=4, space="PSUM"))
            lpool = ctx.enter_context(tc.tile_pool(name="outlo", bufs=1))
            opool = ctx.enter_context(tc.tile_pool(name="osb", bufs=3))

            # --- constants / per-core edge data, loaded once ---
            # iota must precede load_library(mlp): InstIota lives in the
            # default library overlay.
            iota_t = epool.tile([P, Cmax * P], bf16)
            nc.gpsimd.iota(
                iota_t[:],
                pattern=[[0, Cmax], [1, P]],
                base=0,
                channel_multiplier=0,
                allow_small_or_imprecise_dtypes=True,
            )
            nc.gpsimd.load_library(mlp)
            # Preallocate the gather-count registers now: to_reg's scratch
            # pool is exhausted once the full phase-1 program has been
            # emitted, so late allocation fails at this problem size.
            # The SWDGE ring rejects >1024 descriptors per instruction, so
            # gathers are split into runs of <= GMAX chunks.
            GMAX = 8
            sizes = set()
            for cn in (C_lo, C_hi):
                off = 0
                while off < cn:
                    sizes.add(min(GMAX, cn - off))
                    off += min(GMAX, cn - off)
            gregs = {g: nc.gpsimd.to_reg(g * P) for g in sorted(sizes)}
            w_t = wpool.tile([P, KD * d_out], bf16)
            for k in range(KD):
                nc.sync.dma_start(
                    w_t[:, k * d_out : (k + 1) * d_out], Wp[k * P : (k + 1) * P, :]
                )
            idx_t = epool.tile([P, blocks * C * 8], i16)
            dst_t = epool.tile([P, blocks * C], bf16)
            val_t = epool.tile([P, blocks * C], bf16)
            nc.sync.dma_start(idx_t[:], idxp[:])
            nc.sync.dma_start(dst_t[:], dstp[:])
            nc.sync.dma_start(val_t[:], valp[:])

            # --- phase 1: h = x @ W (bf16, replicated), low half first ---
            SBN = strip_blocks * P  # nodes per strip
            for hdst, nb_half, col0 in ((h_lo, NB_lo, 0), (h_hi, NB_hi, H)):
                for s0 in range(0, nb_half, strip_blocks):
                    nbc = min(strip_blocks, nb_half - s0)
                    sn = nbc * P
                    xs = xpool.tile([P, KD * SBN], bf16, tag="xs")
                    for k in range(KD):
                        nc.sync.dma_start(
                            xs[:, k * SBN : k * SBN + sn],
                            xT[k * P : (k + 1) * P, col0 + s0 * P : col0 + s0 * P + sn],
                        )
                    hs = hpool.tile([P, strip_blocks * d_out], bf16, tag="hs")
                    for j in range(nbc):
                        pt = ps1.tile([P, d_out], f32)
                        for k in range(KD):
                            nc.tensor.matmul(
                                pt[:],
                                lhsT=xs[:, k * SBN + j * P : k * SBN + (j + 1) * P],
                                rhs=w_t[:, k * d_out : (k + 1) * d_out],
                                start=(k == 0),
                                stop=(k == KD - 1),
                            )
                        nc.vector.tensor_copy(hs[:, j * d_out : (j + 1) * d_out], pt[:])
                    hd = hdst[s0 * P : s0 * P + sn, :].rearrange(
                        "(nb p) f -> p nb f", p=P
                    )
                    hsv = hs[:, : nbc * d_out].rearrange("p (nb f) -> p nb f", nb=nbc)
                    nc.sync.dma_start(hd, hsv)

            # --- phase 2: batched gather + one-hot scatter matmul per block ---
            out_lo = lpool.tile([P, blocks * d_out], f32)

            def half_pass(b, hsrc, c0, cn, qn):
                """Gather cn chunks (cols c0..c0+cn of block b's edge arrays)
                from hsrc; one-hot scatter-matmul them into a PSUM acc."""
                msgs = mpool.tile([P, cn * d_out], bf16, tag="msgs")
                off = 0
                while off < cn:
                    g = min(GMAX, cn - off)
                    nc.gpsimd.dma_gather(
                        msgs[:, off * d_out : (off + g) * d_out].rearrange(
                            "p (c f) -> p c f", f=d_out
                        ),
                        hsrc[:],
                        idx_t[:, (b * C + c0 + off) * 8 : (b * C + c0 + off + g) * 8],
                        g * P,
                        gregs[g],
                        d_out,
                        queue_num=qn,
                    )
                    off += g
                pt3 = ppool.tile([P, cn * P], bf16, tag="P")
                iota3 = bass.AP(
                    iota_t[:].tensor, iota_t[:].offset,
                    [iota_t[:].ap[0], [P, cn], [1, P]],
                )
                p3 = bass.AP(
                    pt3[:].tensor, pt3[:].offset, [pt3[:].ap[0], [P, cn], [1, P]]
                )
                dstb = dst_t[:, b * C + c0 : b * C + c0 + cn]
                valb = val_t[:, b * C + c0 : b * C + c0 + cn]
                dst_bc = bass.AP(dstb.tensor, dstb.offset, dstb.ap + [[0, P]])
                val_bc = bass.AP(valb.tensor, valb.offset, valb.ap + [[0, P]])
                nc.vector.tensor_tensor(
                    out=p3, in0=iota3, in1=dst_bc, op=mybir.AluOpType.is_equal
                )
                nc.vector.tensor_tensor(
                    out=p3, in0=p3, in1=val_bc, op=mybir.AluOpType.mult
                )
                acc = ps2.tile([P, d_out], f32)
                for c in range(cn):
                    nc.tensor.matmul(
                        acc[:],
                        lhsT=pt3[:, c * P : (c + 1) * P],
                        rhs=msgs[:, c * d_out : (c + 1) * d_out],
                        start=(c == 0),
                        stop=(c == cn - 1),
                    )
                return acc

            for b in range(blocks):  # pass A: low-half src
                acc = half_pass(b, h_lo, 0, C_lo, b % 4)
                nc.vector.tensor_copy(out_lo[:, b * d_out : (b + 1) * d_out], acc[:])
            for b in range(blocks):  # pass B: high-half src + combine
                acc = half_pass(b, h_hi, C_lo, C_hi, (b + 2) % 4)
                ot = opool.tile([P, d_out], f32)
                nc.vector.tensor_tensor(
                    out=ot[:],
                    in0=acc[:],
                    in1=out_lo[:, b * d_out : (b + 1) * d_out],
                    op=mybir.AluOpType.add,
                )
                ot2 = opool.tile([P, d_out], f32)
                nc.scalar.activation(ot2[:], ot[:], mybir.ActivationFunctionType.Relu)
                nc.sync.dma_start(outp[b * P : (b + 1) * P, :], ot2[:])
    lower_extended_insts(nc)
    _legalize_sync_waits(nc)
    return nc


def _pack_edges(edge_src, edge_dst, edge_vals, n_nodes, blocks, H):
    """Permute dst nodes into degree-balanced blocks of 128; split each
    block's edges by src half (< H vs >= H); pack into per-core arrays:
    int16 gather indices (16-partition-wrapped, replicated to 128) plus
    dst-slot/val arrays in [lane, block*C+chunk] layout."""
    import heapq

    total_blocks = N_CORES * blocks
    deg = np.bincount(edge_dst, minlength=n_nodes).astype(np.int64)
    order = np.argsort(-deg, kind="stable")
    heap = [(0, b) for b in range(total_blocks)]
    heapq.heapify(heap)
    counts = np.zeros(total_blocks, np.int32)
    loads = np.zeros(total_blocks, np.int64)
    block_of = np.empty(n_nodes, np.int32)
    slot_of = np.empty(n_nodes, np.int32)
    for node in order:
        while True:
            load, b = heapq.heappop(heap)
            if counts[b] < P:
                break
        block_of[node] = b
        slot_of[node] = counts[b]
        counts[b] += 1
        loads[b] = load + deg[node]
        if counts[b] < P:
            heapq.heappush(heap, (loads[b], b))

    E = len(edge_src)
    eb = block_of[edge_dst].astype(np.int64)
    ishi = (edge_src >= H).astype(np.int64)
    g = eb * 2 + ishi
    eorder = np.argsort(g, kind="stable")
    gs = g[eorder]
    gcounts = np.bincount(gs, minlength=total_blocks * 2)
    C_lo = max(1, math.ceil(gcounts[0::2].max() / P))
    C_hi = max(1, math.ceil(gcounts[1::2].max() / P))
    C = C_lo + C_hi
    goffs = np.concatenate([[0], np.cumsum(gcounts)[:-1]])
    pos = np.arange(E) - goffs[gs]
    ch = np.where(gs % 2 == 0, 0, C_lo) + (pos >> 7)
    lane = pos & 127
    blk = gs >> 1

    idx_a = np.zeros((total_blocks, C, P), np.int16)
    dst_a = np.zeros((total_blocks, C, P), np.float32)
    val_a = np.zeros((total_blocks, C, P), np.float32)
    src_o = edge_src[eorder]
    idx_a[blk, ch, lane] = np.where(src_o < H, src_o, src_o - H).astype(np.int16)
    dst_a[blk, ch, lane] = slot_of[edge_dst[eorder]]
    val_a[blk, ch, lane] = edge_vals[eorder]

    # 16-partition wrap per (block, half): idx j -> (j%16, j//16), chunks
    # flattened chunk-major; lo cols then hi cols; replicate to 128 parts.
    lo_w = idx_a[:, :C_lo].reshape(total_blocks, -1, 16).transpose(0, 2, 1)
    hi_w = idx_a[:, C_lo:].reshape(total_blocks, -1, 16).transpose(0, 2, 1)
    idx_w = np.concatenate([lo_w, hi_w], axis=2)  # [tb, 16, C*8]
    bf = ml_dtypes.bfloat16

    idx_c, dst_c, val_c = [], [], []
    for m in range(N_CORES):
        iw = idx_w[m * blocks : (m + 1) * blocks]  # [blocks, 16, C*8]
        iw = iw.transpose(1, 0, 2).reshape(16, -1)  # [16, blocks*C*8]
        idx_c.append(np.ascontiguousarray(np.tile(iw, (8, 1))))
        for a, dst in ((dst_a, dst_c), (val_a, val_c)):
            t = a[m * blocks : (m + 1) * blocks].transpose(2, 0, 1).reshape(P, -1)
            dst.append(np.ascontiguousarray(t.astype(bf)))
    return idx_c, dst_c, val_c, block_of, slot_of, C_lo, C_hi


def _run(x, W, edge_vals, edge_src, edge_dst, blocks=None, trace=False):
    n_nodes, d_in = x.shape
    d_out = W.shape[1]
    if blocks is None:
        blocks = math.ceil(n_nodes / (N_CORES * P))
    n_pad = math.ceil(n_nodes / P) * P
    H = (math.ceil(n_pad / P / 2)) * P  # low-half rows; < 32768 for int16
    assert H < 32768 and (n_pad - H) < 32768
    idx_c, dst_c, val_c, block_of, slot_of, C_lo, C_hi = _pack_edges(
        edge_src, edge_dst, edge_vals, n_nodes, blocks, H
    )
    bf = ml_dtypes.bfloat16
    xp = np.zeros((n_pad, d_in), np.float32)
    xp[:n_nodes] = x
    xT = np.ascontiguousarray(xp.T.astype(bf))
    Wb = np.ascontiguousarray(W.astype(bf))

    nc = build_nc(n_pad, H, d_in, d_out, blocks, C_lo, C_hi)
    in_maps = [
        {"xT": xT, "W": Wb, "idx": idx_c[m], "dstv": dst_c[m], "valv": val_c[m]}
        for m in range(N_CORES)
    ]
    res = run_bass_kernel_spmd(nc, in_maps, list(range(N_CORES)), trace=trace)
    stacked = np.concatenate([res.results[m]["out"] for m in range(N_CORES)], axis=0)
    npc = blocks * P
    gidx = (block_of // blocks) * npc + (block_of % blocks) * P + slot_of
    out = stacked[gidx]
    return out, res


def kernel(x, W, edge_vals, edge_src, edge_dst):
    x = np.asarray(x, np.float32)
    W = np.asarray(W, np.float32)
    edge_vals = np.asarray(edge_vals, np.float32)
    edge_src = np.asarray(edge_src).astype(np.int64)
    edge_dst = np.asarray(edge_dst).astype(np.int64)
    out, _ = _run(x, W, edge_vals, edge_src, edge_dst)
    return out.astype(np.float32)


# revision 7
# speedup vs baseline: 2.2342x; 1.2035x over previous
"""GNN message-passing layer (out = relu(segment_sum(vals * (xW)[src] by dst)))
on 8 Trainium2 NeuronCores.

Strategy (1D graph partitioning, per sharding hint):
- dst nodes are permuted into 8*BLOCKS blocks of 128, degree-balanced so
  every block has bounded incoming edges; core m owns blocks
  [m*BLOCKS, (m+1)*BLOCKS) and produces those output rows.
- Every core computes the full dense transform h = x @ W in bf16 (replicated;
  avoids cross-core communication), written as two DRAM tensors h_lo/h_hi
  (node halves) so phase 2's low-half gathers can start while the high half
  is still being computed.
- Per dst block and src half: one batched SWDGE dma_gather pulls all C*128
  source rows h[src] into SBUF in [lane, chunk, feat] layout (idx i ->
  partition i%128, chunk i//128), amortizing the ~1us fixed SWDGE cost over
  the whole block instead of paying it per 128-row chunk. int16 gather
  indices only span 32K rows, hence the lo/hi split.
- A value-scaled one-hot matrix P (DVE iota/is_equal, bf16) is matmul'd
  against the messages, accumulating into PSUM:
  psum[d, f] = sum_e val[e] * [dst_e == d] * h[src_e, f]; low-half partial
  sums park in SBUF f32; high-half pass adds, relu, store.
- Padding edges carry val = 0 and gather row 0, so they contribute nothing.
"""
import math
from contextlib import ExitStack

import numpy as np
import ml_dtypes

import concourse.bass as bass
import concourse.mybir as mybir
import concourse.tile as tile
from concourse.bass_utils import run_bass_kernel_spmd
from concourse.library_config import mlp
from concourse.library_overlay import lower_extended_insts
from concourse.vector_clock import ScopedClock

# --- workaround: this walrus build rejects >1 sync wait per instruction
# ("Too many sync wait commands"). Tile's kernel-tail drain carries one wait
# per active sem lane; give it the same NOP-splitting treatment as everything
# else via a post-schedule legalization pass over all basic blocks. ---
_MAX_WAITS = 1


def _patched_drain_and_barrier(self, tick_clock, wait_clock):
    drain_inst = self.nc.sync.drain()
    wait_clock.add_sem_waits(
        drain_inst.ins, ScopedClock({None: tick_clock.global_clock})
    )
    self.nc.all_engine_barrier()
    popped = self.nc._tile_sem_poison_stack.pop()
    assert popped is self._sem_poison
    self.nc.clear_and_free_semaphores(list(self.sems.allocated().values()))
    self.nc.all_engine_barrier()


tile.TileContext._drain_and_barrier = _patched_drain_and_barrier


def _legalize_sync_waits(nc):
    """Split instructions carrying >_MAX_WAITS sem waits: excess waits move to
    same-engine NOPs inserted immediately before the instruction."""
    n_split = 0
    for f in nc.m.functions:
        for bb in f.blocks:
            out = []
            changed = False
            for ins in bb.instructions:
                si = ins.sync_info
                waits = list(si.on_wait) if si and si.on_wait else []
                if len(waits) > _MAX_WAITS:
                    changed = True
                    n_split += 1
                    for i in range(_MAX_WAITS, len(waits), _MAX_WAITS):
                        nop = mybir.InstNoOp(
                            name=nc.get_next_instruction_name(), ins=[], outs=[]
                        )
                        nop.engine = ins.engine
                        nop.sync_info = mybir.SyncInfo(
                            on_wait=waits[i : i + _MAX_WAITS], on_update=[]
                        )
                        nc.register_instruction(nop, overwrite=True)
                        out.append(nop)
                    si.on_wait = waits[:_MAX_WAITS]
                out.append(ins)
            if changed:
                bb.instructions = out
    return n_split

N_CORES = 8
P = 128


def build_nc(n_pad, H, d_in, d_out, blocks, C_lo, C_hi, strip_blocks=8):
    """One SPMD program. n_pad: padded node count (mult of 128), H: low-half
    row count (mult of 128, < 32768). blocks: dst blocks per core. C_lo/C_hi:
    chunks (of 128 edges) per block for the low/high src halves."""
    f32 = mybir.dt.float32
    bf16 = mybir.dt.bfloat16
    i16 = mybir.dt.int16
    KD = d_in // P
    C = C_lo + C_hi
    Cmax = max(C_lo, C_hi)
    NB_lo = H // P
    NB_hi = (n_pad - H) // P

    nc = bass.Bass(num_swdge_queues=4)
    xT = nc.declare_dram_parameter("xT", [d_in, n_pad], bf16, isOutput=False)
    Wp = nc.declare_dram_parameter("W", [d_in, d_out], bf16, isOutput=False)
    idxp = nc.declare_dram_parameter("idx", [P, blocks * C * 8], i16, isOutput=False)
    dstp = nc.declare_dram_parameter("dstv", [P, blocks * C], bf16, isOutput=False)
    valp = nc.declare_dram_parameter("valv", [P, blocks * C], bf16, isOutput=False)
    outp = nc.declare_dram_parameter("out", [blocks * P, d_out], f32, isOutput=True)
    h_lo = nc.dram_tensor("h_lo", [H, d_out], bf16)
    h_hi = nc.dram_tensor("h_hi", [n_pad - H, d_out], bf16)

    with tile.TileContext(nc) as tc:
        with ExitStack() as ctx:
            wpool = ctx.enter_context(tc.tile_pool(name="w", bufs=1))
            epool = ctx.enter_context(tc.tile_pool(name="edges", bufs=1))
            xpool = ctx.enter_context(tc.tile_pool(name="xs", bufs=2))
            hpool = ctx.enter_context(tc.tile_pool(name="hs", bufs=2))
            ps1 = ctx.enter_context(tc.tile_pool(name="ps1", bufs=4, space="PSUM"))
            mpool = ctx.enter_context(tc.tile_pool(name="msgs", bufs=4))
            ppool = ctx.enter_context(tc.tile_pool(name="onehot", bufs=3))
            ps2 = ctx.enter_context(tc.tile_pool(name="ps2", bufs=4, space="PSUM"))
            lpool = ctx.enter_context(tc.tile_pool(name="outlo", bufs=1))
            opool = ctx.enter_context(tc.tile_pool(name="osb", bufs=3))

            # --- constants / per-core edge data, loaded once ---
            # iota must precede load_library(mlp): InstIota lives in the
            # default library overlay.
            iota_t = epool.tile([P, Cmax * P], bf16)
            nc.gpsimd.iota(
                iota_t[:],
                pattern=[[0, Cmax], [1, P]],
                base=0,
                channel_multiplier=0,
                allow_small_or_imprecise_dtypes=True,
            )
            nc.gpsimd.load_library(mlp)
            # Preallocate the gather-count registers now: to_reg's scratch
            # pool is exhausted once the full phase-1 program has been
            # emitted, so late allocation fails at this problem size.
            # The SWDGE ring rejects >1024 descriptors per instruction, so
            # gathers are split into runs of <= GMAX chunks.
            GMAX = 8
            sizes = set()
            for cn in (C_lo, C_hi):
                off = 0
                while off < cn:
                    sizes.add(min(GMAX, cn - off))
                    off += min(GMAX, cn - off)
            gregs = {g: nc.gpsimd.to_reg(g * P) for g in sorted(sizes)}
            w_t = wpool.tile([P, KD * d_out], bf16)
            for k in range(KD):
                nc.sync.dma_start(
                    w_t[:, k * d_out : (k + 1) * d_out], Wp[k * P : (k + 1) * P, :]
                )
            idx_t = epool.tile([P, blocks * C * 8], i16)
            dst_t = epool.tile([P, blocks * C], bf16)
            val_t = epool.tile([P, blocks * C], bf16)
            nc.sync.dma_start(idx_t[:], idxp[:])
            nc.sync.dma_start(dst_t[:], dstp[:])
            nc.sync.dma_start(val_t[:], valp[:])

            # --- phase 1: h = x @ W (bf16, replicated), low half first ---
            SBN = strip_blocks * P  # nodes per strip
            for hdst, nb_half, col0 in ((h_lo, NB_lo, 0), (h_hi, NB_hi, H)):
                for s0 in range(0, nb_half, strip_blocks):
                    nbc = min(strip_blocks, nb_half - s0)
                    sn = nbc * P
                    xs = xpool.tile([P, KD * SBN], bf16, tag="xs")
                    for k in range(KD):
                        nc.sync.dma_start(
                            xs[:, k * SBN : k * SBN + sn],
                            xT[k * P : (k + 1) * P, col0 + s0 * P : col0 + s0 * P + sn],
                        )
                    hs = hpool.tile([P, strip_blocks * d_out], bf16, tag="hs")
                    for j in range(nbc):
                        pt = ps1.tile([P, d_out], f32)
                        for k in range(KD):
                            nc.tensor.matmul(
                                pt[:],
                                lhsT=xs[:, k * SBN + j * P : k * SBN + (j + 1) * P],
                                rhs=w_t[:, k * d_out : (k + 1) * d_out],
                                start=(k == 0),
                                stop=(k == KD - 1),
                            )
                        nc.vector.tensor_copy(hs[:, j * d_out : (j + 1) * d_out], pt[:])
                    hd = hdst[s0 * P : s0 * P + sn, :].rearrange(
                        "(nb p) f -> p nb f", p=P
                    )
                    hsv = hs[:, : nbc * d_out].rearrange("p (nb f) -> p nb f", nb=nbc)
                    nc.sync.dma_start(hd, hsv)

            # --- phase 2: batched gather + one-hot scatter matmul per block ---
            out_lo = lpool.tile([P, blocks * d_out], f32)

            qctr = [0]

            def half_pass(b, hsrc, c0, cn):
                """Gather cn chunks (cols c0..c0+cn of block b's edge arrays)
                from hsrc; one-hot scatter-matmul them into a PSUM acc."""
                msgs = mpool.tile([P, cn * d_out], bf16, tag="msgs")
                off = 0
                while off < cn:
                    g = min(GMAX, cn - off)
                    nc.gpsimd.dma_gather(
                        msgs[:, off * d_out : (off + g) * d_out].rearrange(
                            "p (c f) -> p c f", f=d_out
                        ),
                        hsrc[:],
                        idx_t[:, (b * C + c0 + off) * 8 : (b * C + c0 + off + g) * 8],
                        g * P,
                        gregs[g],
                        d_out,
                        queue_num=qctr[0] % 4,
                    )
                    qctr[0] += 1
                    off += g
                pt3 = ppool.tile([P, cn * P], bf16, tag="P")
                iota3 = bass.AP(
                    iota_t[:].tensor, iota_t[:].offset,
                    [iota_t[:].ap[0], [P, cn], [1, P]],
                )
                p3 = bass.AP(
                    pt3[:].tensor, pt3[:].offset, [pt3[:].ap[0], [P, cn], [1, P]]
                )
                dstb = dst_t[:, b * C + c0 : b * C + c0 + cn]
                valb = val_t[:, b * C + c0 : b * C + c0 + cn]
                dst_bc = bass.AP(dstb.tensor, dstb.offset, dstb.ap + [[0, P]])
                val_bc = bass.AP(valb.tensor, valb.offset, valb.ap + [[0, P]])
                nc.vector.tensor_tensor(
                    out=p3, in0=iota3, in1=dst_bc, op=mybir.AluOpType.is_equal
                )
                nc.vector.tensor_tensor(
                    out=p3, in0=p3, in1=val_bc, op=mybir.AluOpType.mult
                )
                acc = ps2.tile([P, d_out], f32)
                for c in range(cn):
                    nc.tensor.matmul(
                        acc[:],
                        lhsT=pt3[:, c * P : (c + 1) * P],
                        rhs=msgs[:, c * d_out : (c + 1) * d_out],
                        start=(c == 0),
                        stop=(c == cn - 1),
                    )
                return acc

            for b in range(blocks):  # pass A: low-half src
                acc = half_pass(b, h_lo, 0, C_lo)
                nc.vector.tensor_copy(out_lo[:, b * d_out : (b + 1) * d_out], acc[:])
            for b in range(blocks):  # pass B: high-half src + combine
                acc = half_pass(b, h_hi, C_lo, C_hi)
                ot = opool.tile([P, d_out], f32)
                nc.vector.tensor_tensor(
                    out=ot[:],
                    in0=acc[:],
                    in1=out_lo[:, b * d_out : (b + 1) * d_out],
                    op=mybir.AluOpType.add,
                )
                ot2 = opool.tile([P, d_out], f32)
                nc.scalar.activation(ot2[:], ot[:], mybir.ActivationFunctionType.Relu)
                nc.sync.dma_start(outp[b * P : (b + 1) * P, :], ot2[:])
    lower_extended_insts(nc)
    _legalize_sync_waits(nc)
    return nc


def _pack_edges(edge_src, edge_dst, edge_vals, n_nodes, blocks, H):
    """Permute dst nodes into degree-balanced blocks of 128; split each
    block's edges by src half (< H vs >= H); pack into per-core arrays:
    int16 gather indices (16-partition-wrapped, replicated to 128) plus
    dst-slot/val arrays in [lane, block*C+chunk] layout."""
    import heapq

    total_blocks = N_CORES * blocks
    deg = np.bincount(edge_dst, minlength=n_nodes).astype(np.int64)
    order = np.argsort(-deg, kind="stable")
    heap = [(0, b) for b in range(total_blocks)]
    heapq.heapify(heap)
    counts = np.zeros(total_blocks, np.int32)
    loads = np.zeros(total_blocks, np.int64)
    block_of = np.empty(n_nodes, np.int32)
    slot_of = np.empty(n_nodes, np.int32)
    for node in order:
        while True:
            load, b = heapq.heappop(heap)
            if counts[b] < P:
                break
        block_of[node] = b
        slot_of[node] = counts[b]
        counts[b] += 1
        loads[b] = load + deg[node]
        if counts[b] < P:
            heapq.heappush(heap, (loads[b], b))

    E = len(edge_src)
    eb = block_of[edge_dst].astype(np.int64)
    ishi = (edge_src >= H).astype(np.int64)
    g = eb * 2 + ishi
    eorder = np.argsort(g, kind="stable")
    gs = g[eorder]
    gcounts = np.bincount(gs, minlength=total_blocks * 2)
    C_lo = max(1, math.ceil(gcounts[0::2].max() / P))
    C_hi = max(1, math.ceil(gcounts[1::2].max() / P))
    C = C_lo + C_hi
    goffs = np.concatenate([[0], np.cumsum(gcounts)[:-1]])
    pos = np.arange(E) - goffs[gs]
    ch = np.where(gs % 2 == 0, 0, C_lo) + (pos >> 7)
    lane = pos & 127
    blk = gs >> 1

    idx_a = np.zeros((total_blocks, C, P), np.int16)
    dst_a = np.zeros((total_blocks, C, P), np.float32)
    val_a = np.zeros((total_blocks, C, P), np.float32)
    src_o = edge_src[eorder]
    idx_a[blk, ch, lane] = np.where(src_o < H, src_o, src_o - H).astype(np.int16)
    dst_a[blk, ch, lane] = slot_of[edge_dst[eorder]]
    val_a[blk, ch, lane] = edge_vals[eorder]

    # 16-partition wrap per (block, half): idx j -> (j%16, j//16), chunks
    # flattened chunk-major; lo cols then hi cols; replicate to 128 parts.
    lo_w = idx_a[:, :C_lo].reshape(total_blocks, -1, 16).transpose(0, 2, 1)
    hi_w = idx_a[:, C_lo:].reshape(total_blocks, -1, 16).transpose(0, 2, 1)
    idx_w = np.concatenate([lo_w, hi_w], axis=2)  # [tb, 16, C*8]
    bf = ml_dtypes.bfloat16

    idx_c, dst_c, val_c = [], [], []
    for m in range(N_CORES):
        iw = idx_w[m * blocks : (m + 1) * blocks]  # [blocks, 16, C*8]
        iw = iw.transpose(1, 0, 2).reshape(16, -1)  # [16, blocks*C*8]
        idx_c.append(np.ascontiguousarray(np.tile(iw, (8, 1))))
        for a, dst in ((dst_a, dst_c), (val_a, val_c)):
            t = a[m * blocks : (m + 1) * blocks].transpose(2, 0, 1).reshape(P, -1)
            dst.append(np.ascontiguousarray(t.astype(bf)))
    return idx_c, dst_c, val_c, block_of, slot_of, C_lo, C_hi


def _run(x, W, edge_vals, edge_src, edge_dst, blocks=None, trace=False):
    n_nodes, d_in = x.shape
    d_out = W.shape[1]
    if blocks is None:
        blocks = math.ceil(n_nodes / (N_CORES * P))
    n_pad = math.ceil(n_nodes / P) * P
    H = (math.ceil(n_pad / P / 2)) * P  # low-half rows; < 32768 for int16
    assert H < 32768 and (n_pad - H) < 32768
    idx_c, dst_c, val_c, block_of, slot_of, C_lo, C_hi = _pack_edges(
        edge_src, edge_dst, edge_vals, n_nodes, blocks, H
    )
    bf = ml_dtypes.bfloat16
    xp = np.zeros((n_pad, d_in), np.float32)
    xp[:n_nodes] = x
    xT = np.ascontiguousarray(xp.T.astype(bf))
    Wb = np.ascontiguousarray(W.astype(bf))

    nc = build_nc(n_pad, H, d_in, d_out, blocks, C_lo, C_hi)
    in_maps = [
        {"xT": xT, "W": Wb, "idx": idx_c[m], "dstv": dst_c[m], "valv": val_c[m]}
        for m in range(N_CORES)
    ]
    res = run_bass_kernel_spmd(nc, in_maps, list(range(N_CORES)), trace=trace)
    stacked = np.concatenate([res.results[m]["out"] for m in range(N_CORES)], axis=0)
    npc = blocks * P
    gidx = (block_of // blocks) * npc + (block_of % blocks) * P + slot_of
    out = stacked[gidx]
    return out, res


def kernel(x, W, edge_vals, edge_src, edge_dst):
    x = np.asarray(x, np.float32)
    W = np.asarray(W, np.float32)
    edge_vals = np.asarray(edge_vals, np.float32)
    edge_src = np.asarray(edge_src).astype(np.int64)
    edge_dst = np.asarray(edge_dst).astype(np.int64)
    out, _ = _run(x, W, edge_vals, edge_src, edge_dst)
    return out.astype(np.float32)


# revision 8
# speedup vs baseline: 2.3549x; 1.0540x over previous
"""GNN message-passing layer (out = relu(segment_sum(vals * (xW)[src] by dst)))
on 8 Trainium2 NeuronCores.

Strategy (1D graph partitioning, per sharding hint):
- dst nodes are permuted into 8*BLOCKS blocks of 128, degree-balanced so
  every block has bounded incoming edges; core m owns blocks
  [m*BLOCKS, (m+1)*BLOCKS) and produces those output rows.
- Every core computes the full dense transform h = x @ W in bf16 (replicated;
  avoids cross-core communication), written as two DRAM tensors h_lo/h_hi
  (node halves) so phase 2's low-half gathers can start while the high half
  is still being computed.
- Per dst block and src half: one batched SWDGE dma_gather pulls all C*128
  source rows h[src] into SBUF in [lane, chunk, feat] layout (idx i ->
  partition i%128, chunk i//128), amortizing the ~1us fixed SWDGE cost over
  the whole block instead of paying it per 128-row chunk. int16 gather
  indices only span 32K rows, hence the lo/hi split.
- A value-scaled one-hot matrix P (DVE iota/is_equal, bf16) is matmul'd
  against the messages, accumulating into PSUM:
  psum[d, f] = sum_e val[e] * [dst_e == d] * h[src_e, f]; low-half partial
  sums park in SBUF f32; high-half pass adds, relu, store.
- Padding edges carry val = 0 and gather row 0, so they contribute nothing.
"""
import math
from contextlib import ExitStack

import numpy as np
import ml_dtypes

import concourse.bass as bass
import concourse.mybir as mybir
import concourse.tile as tile
from concourse.bass_utils import run_bass_kernel_spmd
from concourse.library_config import mlp
from concourse.library_overlay import lower_extended_insts
from concourse.vector_clock import ScopedClock

# --- workaround: this walrus build rejects >1 sync wait per instruction
# ("Too many sync wait commands"). Tile's kernel-tail drain carries one wait
# per active sem lane; give it the same NOP-splitting treatment as everything
# else via a post-schedule legalization pass over all basic blocks. ---
_MAX_WAITS = 1


def _patched_drain_and_barrier(self, tick_clock, wait_clock):
    drain_inst = self.nc.sync.drain()
    wait_clock.add_sem_waits(
        drain_inst.ins, ScopedClock({None: tick_clock.global_clock})
    )
    self.nc.all_engine_barrier()
    popped = self.nc._tile_sem_poison_stack.pop()
    assert popped is self._sem_poison
    self.nc.clear_and_free_semaphores(list(self.sems.allocated().values()))
    self.nc.all_engine_barrier()


tile.TileContext._drain_and_barrier = _patched_drain_and_barrier


def _legalize_sync_waits(nc):
    """Split instructions carrying >_MAX_WAITS sem waits: excess waits move to
    same-engine NOPs inserted immediately before the instruction."""
    n_split = 0
    for f in nc.m.functions:
        for bb in f.blocks:
            out = []
            changed = False
            for ins in bb.instructions:
                si = ins.sync_info
                waits = list(si.on_wait) if si and si.on_wait else []
                if len(waits) > _MAX_WAITS:
                    changed = True
                    n_split += 1
                    for i in range(_MAX_WAITS, len(waits), _MAX_WAITS):
                        nop = mybir.InstNoOp(
                            name=nc.get_next_instruction_name(), ins=[], outs=[]
                        )
                        nop.engine = ins.engine
                        nop.sync_info = mybir.SyncInfo(
                            on_wait=waits[i : i + _MAX_WAITS], on_update=[]
                        )
                        nc.register_instruction(nop, overwrite=True)
                        out.append(nop)
                    si.on_wait = waits[:_MAX_WAITS]
                out.append(ins)
            if changed:
                bb.instructions = out
    return n_split

N_CORES = 8
P = 128


def build_nc(n_pad, H, d_in, d_out, blocks, C_lo, C_hi, strip_blocks=8):
    """One SPMD program. n_pad: padded node count (mult of 128), H: low-half
    row count (mult of 128, < 32768). blocks: dst blocks per core. C_lo/C_hi:
    chunks (of 128 edges) per block for the low/high src halves."""
    f32 = mybir.dt.float32
    bf16 = mybir.dt.bfloat16
    i16 = mybir.dt.int16
    KD = d_in // P
    C = C_lo + C_hi
    Cmax = max(C_lo, C_hi)
    NB_lo = H // P
    NB_hi = (n_pad - H) // P

    nc = bass.Bass(num_swdge_queues=4)
    xT = nc.declare_dram_parameter("xT", [d_in, n_pad], bf16, isOutput=False)
    Wp = nc.declare_dram_parameter("W", [d_in, d_out], bf16, isOutput=False)
    idxp = nc.declare_dram_parameter("idx", [P, blocks * C * 8], i16, isOutput=False)
    dstp = nc.declare_dram_parameter("dstv", [P, blocks * C], bf16, isOutput=False)
    valp = nc.declare_dram_parameter("valv", [P, blocks * C], bf16, isOutput=False)
    outp = nc.declare_dram_parameter("out", [blocks * P, d_out], f32, isOutput=True)
    h_lo = nc.dram_tensor("h_lo", [H, d_out], bf16)
    h_hi = nc.dram_tensor("h_hi", [n_pad - H, d_out], bf16)

    with tile.TileContext(nc) as tc:
        with ExitStack() as ctx:
            wpool = ctx.enter_context(tc.tile_pool(name="w", bufs=1))
            epool = ctx.enter_context(tc.tile_pool(name="edges", bufs=1))
            xpool = ctx.enter_context(tc.tile_pool(name="xs", bufs=2))
            hpool = ctx.enter_context(tc.tile_pool(name="hs", bufs=2))
            ps1 = ctx.enter_context(tc.tile_pool(name="ps1", bufs=4, space="PSUM"))
            mpool = ctx.enter_context(tc.tile_pool(name="msgs", bufs=6))
            ppool = ctx.enter_context(tc.tile_pool(name="onehot", bufs=4))
            ps2 = ctx.enter_context(tc.tile_pool(name="ps2", bufs=4, space="PSUM"))
            lpool = ctx.enter_context(tc.tile_pool(name="outlo", bufs=1))
            opool = ctx.enter_context(tc.tile_pool(name="osb", bufs=3))

            # --- constants / per-core edge data, loaded once ---
            # iota must precede load_library(mlp): InstIota lives in the
            # default library overlay.
            iota_t = epool.tile([P, Cmax * P], bf16)
            nc.gpsimd.iota(
                iota_t[:],
                pattern=[[0, Cmax], [1, P]],
                base=0,
                channel_multiplier=0,
                allow_small_or_imprecise_dtypes=True,
            )
            nc.gpsimd.load_library(mlp)
            # Preallocate the gather-count registers now: to_reg's scratch
            # pool is exhausted once the full phase-1 program has been
            # emitted, so late allocation fails at this problem size.
            # The SWDGE ring rejects >1024 descriptors per instruction, so
            # gathers are split into runs of <= GMAX chunks.
            GMAX = 8
            sizes = set()
            for cn in (C_lo, C_hi):
                off = 0
                while off < cn:
                    sizes.add(min(GMAX, cn - off))
                    off += min(GMAX, cn - off)
            gregs = {g: nc.gpsimd.to_reg(g * P) for g in sorted(sizes)}
            w_t = wpool.tile([P, KD * d_out], bf16)
            for k in range(KD):
                nc.sync.dma_start(
                    w_t[:, k * d_out : (k + 1) * d_out], Wp[k * P : (k + 1) * P, :]
                )
            idx_t = epool.tile([P, blocks * C * 8], i16)
            dst_t = epool.tile([P, blocks * C], bf16)
            val_t = epool.tile([P, blocks * C], bf16)
            nc.sync.dma_start(idx_t[:], idxp[:])
            nc.sync.dma_start(dst_t[:], dstp[:])
            nc.sync.dma_start(val_t[:], valp[:])

            # --- phase 1: h = x @ W (bf16, replicated), low half first ---
            SBN = strip_blocks * P  # nodes per strip
            for hdst, nb_half, col0 in ((h_lo, NB_lo, 0), (h_hi, NB_hi, H)):
                for s0 in range(0, nb_half, strip_blocks):
                    nbc = min(strip_blocks, nb_half - s0)
                    sn = nbc * P
                    xs = xpool.tile([P, KD * SBN], bf16, tag="xs")
                    for k in range(KD):
                        nc.sync.dma_start(
                            xs[:, k * SBN : k * SBN + sn],
                            xT[k * P : (k + 1) * P, col0 + s0 * P : col0 + s0 * P + sn],
                        )
                    hs = hpool.tile([P, strip_blocks * d_out], bf16, tag="hs")
                    for j in range(nbc):
                        pt = ps1.tile([P, d_out], f32)
                        for k in range(KD):
                            nc.tensor.matmul(
                                pt[:],
                                lhsT=xs[:, k * SBN + j * P : k * SBN + (j + 1) * P],
                                rhs=w_t[:, k * d_out : (k + 1) * d_out],
                                start=(k == 0),
                                stop=(k == KD - 1),
                            )
                        nc.scalar.activation(
                            hs[:, j * d_out : (j + 1) * d_out],
                            pt[:],
                            mybir.ActivationFunctionType.Copy,
                        )
                    hd = hdst[s0 * P : s0 * P + sn, :].rearrange(
                        "(nb p) f -> p nb f", p=P
                    )
                    hsv = hs[:, : nbc * d_out].rearrange("p (nb f) -> p nb f", nb=nbc)
                    nc.sync.dma_start(hd, hsv)

            # --- phase 2: batched gather + one-hot scatter matmul per block ---
            out_lo = lpool.tile([P, blocks * d_out], f32)

            qctr = [0]

            def half_pass(b, hsrc, c0, cn):
                """Gather cn chunks (cols c0..c0+cn of block b's edge arrays)
                from hsrc; one-hot scatter-matmul them into a PSUM acc."""
                msgs = mpool.tile([P, cn * d_out], bf16, tag="msgs")
                off = 0
                while off < cn:
                    g = min(GMAX, cn - off)
                    nc.gpsimd.dma_gather(
                        msgs[:, off * d_out : (off + g) * d_out].rearrange(
                            "p (c f) -> p c f", f=d_out
                        ),
                        hsrc[:],
                        idx_t[:, (b * C + c0 + off) * 8 : (b * C + c0 + off + g) * 8],
                        g * P,
                        gregs[g],
                        d_out,
                        queue_num=qctr[0] % 4,
                    )
                    qctr[0] += 1
                    off += g
                pt3 = ppool.tile([P, cn * P], bf16, tag="P")
                iota3 = bass.AP(
                    iota_t[:].tensor, iota_t[:].offset,
                    [iota_t[:].ap[0], [P, cn], [1, P]],
                )
                p3 = bass.AP(
                    pt3[:].tensor, pt3[:].offset, [pt3[:].ap[0], [P, cn], [1, P]]
                )
                dstb = dst_t[:, b * C + c0 : b * C + c0 + cn]
                valb = val_t[:, b * C + c0 : b * C + c0 + cn]
                dst_bc = bass.AP(dstb.tensor, dstb.offset, dstb.ap + [[0, P]])
                val_bc = bass.AP(valb.tensor, valb.offset, valb.ap + [[0, P]])
                nc.vector.tensor_tensor(
                    out=p3, in0=iota3, in1=dst_bc, op=mybir.AluOpType.is_equal
                )
                nc.vector.tensor_tensor(
                    out=p3, in0=p3, in1=val_bc, op=mybir.AluOpType.mult
                )
                acc = ps2.tile([P, d_out], f32)
                for c in range(cn):
                    nc.tensor.matmul(
                        acc[:],
                        lhsT=pt3[:, c * P : (c + 1) * P],
                        rhs=msgs[:, c * d_out : (c + 1) * d_out],
                        start=(c == 0),
                        stop=(c == cn - 1),
                    )
                return acc

            for b in range(blocks):  # pass A: low-half src
                acc = half_pass(b, h_lo, 0, C_lo)
                nc.vector.tensor_copy(out_lo[:, b * d_out : (b + 1) * d_out], acc[:])
            for b in range(blocks):  # pass B: high-half src + combine
                acc = half_pass(b, h_hi, C_lo, C_hi)
                ot = opool.tile([P, d_out], f32)
                nc.vector.tensor_tensor(
                    out=ot[:],
                    in0=acc[:],
                    in1=out_lo[:, b * d_out : (b + 1) * d_out],
                    op=mybir.AluOpType.add,
                )
                ot2 = opool.tile([P, d_out], f32)
                nc.scalar.activation(ot2[:], ot[:], mybir.ActivationFunctionType.Relu)
                nc.sync.dma_start(outp[b * P : (b + 1) * P, :], ot2[:])
    lower_extended_insts(nc)
    _legalize_sync_waits(nc)
    return nc


def _pack_edges(edge_src, edge_dst, edge_vals, n_nodes, blocks, H):
    """Permute dst nodes into degree-balanced blocks of 128; split each
    block's edges by src half (< H vs >= H); pack into per-core arrays:
    int16 gather indices (16-partition-wrapped, replicated to 128) plus
    dst-slot/val arrays in [lane, block*C+chunk] layout."""
    import heapq

    total_blocks = N_CORES * blocks
    deg = np.bincount(edge_dst, minlength=n_nodes).astype(np.int64)
    order = np.argsort(-deg, kind="stable")
    heap = [(0, b) for b in range(total_blocks)]
    heapq.heapify(heap)
    counts = np.zeros(total_blocks, np.int32)
    loads = np.zeros(total_blocks, np.int64)
    block_of = np.empty(n_nodes, np.int32)
    slot_of = np.empty(n_nodes, np.int32)
    for node in order:
        while True:
            load, b = heapq.heappop(heap)
            if counts[b] < P:
                break
        block_of[node] = b
        slot_of[node] = counts[b]
        counts[b] += 1
        loads[b] = load + deg[node]
        if counts[b] < P:
            heapq.heappush(heap, (loads[b], b))

    E = len(edge_src)
    eb = block_of[edge_dst].astype(np.int64)
    ishi = (edge_src >= H).astype(np.int64)
    g = eb * 2 + ishi
    eorder = np.argsort(g, kind="stable")
    gs = g[eorder]
    gcounts = np.bincount(gs, minlength=total_blocks * 2)
    C_lo = max(1, math.ceil(gcounts[0::2].max() / P))
    C_hi = max(1, math.ceil(gcounts[1::2].max() / P))
    C = C_lo + C_hi
    goffs = np.concatenate([[0], np.cumsum(gcounts)[:-1]])
    pos = np.arange(E) - goffs[gs]
    ch = np.where(gs % 2 == 0, 0, C_lo) + (pos >> 7)
    lane = pos & 127
    blk = gs >> 1

    idx_a = np.zeros((total_blocks, C, P), np.int16)
    dst_a = np.zeros((total_blocks, C, P), np.float32)
    val_a = np.zeros((total_blocks, C, P), np.float32)
    src_o = edge_src[eorder]
    idx_a[blk, ch, lane] = np.where(src_o < H, src_o, src_o - H).astype(np.int16)
    dst_a[blk, ch, lane] = slot_of[edge_dst[eorder]]
    val_a[blk, ch, lane] = edge_vals[eorder]

    # 16-partition wrap per (block, half): idx j -> (j%16, j//16), chunks
    # flattened chunk-major; lo cols then hi cols; replicate to 128 parts.
    lo_w = idx_a[:, :C_lo].reshape(total_blocks, -1, 16).transpose(0, 2, 1)
    hi_w = idx_a[:, C_lo:].reshape(total_blocks, -1, 16).transpose(0, 2, 1)
    idx_w = np.concatenate([lo_w, hi_w], axis=2)  # [tb, 16, C*8]
    bf = ml_dtypes.bfloat16

    idx_c, dst_c, val_c = [], [], []
    for m in range(N_CORES):
        iw = idx_w[m * blocks : (m + 1) * blocks]  # [blocks, 16, C*8]
        iw = iw.transpose(1, 0, 2).reshape(16, -1)  # [16, blocks*C*8]
        idx_c.append(np.ascontiguousarray(np.tile(iw, (8, 1))))
        for a, dst in ((dst_a, dst_c), (val_a, val_c)):
            t = a[m * blocks : (m + 1) * blocks].transpose(2, 0, 1).reshape(P, -1)
            dst.append(np.ascontiguousarray(t.astype(bf)))
    return idx_c, dst_c, val_c, block_of, slot_of, C_lo, C_hi


def _run(x, W, edge_vals, edge_src, edge_dst, blocks=None, trace=False):
    n_nodes, d_in = x.shape
    d_out = W.shape[1]
    if blocks is None:
        blocks = math.ceil(n_nodes / (N_CORES * P))
    n_pad = math.ceil(n_nodes / P) * P
    H = (math.ceil(n_pad / P / 2)) * P  # low-half rows; < 32768 for int16
    assert H < 32768 and (n_pad - H) < 32768
    idx_c, dst_c, val_c, block_of, slot_of, C_lo, C_hi = _pack_edges(
        edge_src, edge_dst, edge_vals, n_nodes, blocks, H
    )
    bf = ml_dtypes.bfloat16
    xp = np.zeros((n_pad, d_in), np.float32)
    xp[:n_nodes] = x
    xT = np.ascontiguousarray(xp.T.astype(bf))
    Wb = np.ascontiguousarray(W.astype(bf))

    nc = build_nc(n_pad, H, d_in, d_out, blocks, C_lo, C_hi)
    in_maps = [
        {"xT": xT, "W": Wb, "idx": idx_c[m], "dstv": dst_c[m], "valv": val_c[m]}
        for m in range(N_CORES)
    ]
    res = run_bass_kernel_spmd(nc, in_maps, list(range(N_CORES)), trace=trace)
    stacked = np.concatenate([res.results[m]["out"] for m in range(N_CORES)], axis=0)
    npc = blocks * P
    gidx = (block_of // blocks) * npc + (block_of % blocks) * P + slot_of
    out = stacked[gidx]
    return out, res


def kernel(x, W, edge_vals, edge_src, edge_dst):
    x = np.asarray(x, np.float32)
    W = np.asarray(W, np.float32)
    edge_vals = np.asarray(edge_vals, np.float32)
    edge_src = np.asarray(edge_src).astype(np.int64)
    edge_dst = np.asarray(edge_dst).astype(np.int64)
    out, _ = _run(x, W, edge_vals, edge_src, edge_dst)
    return out.astype(np.float32)


# revision 9
# speedup vs baseline: 2.4907x; 1.0577x over previous
"""GNN message-passing layer (out = relu(segment_sum(vals * (xW)[src] by dst)))
on 8 Trainium2 NeuronCores.

Strategy (1D graph partitioning, per sharding hint):
- dst nodes are permuted into 8*BLOCKS blocks of 128, degree-balanced so
  every block has bounded incoming edges; core m owns blocks
  [m*BLOCKS, (m+1)*BLOCKS) and produces those output rows.
- Every core computes the full dense transform h = x @ W in bf16 (replicated;
  avoids cross-core communication), written as two DRAM tensors h_lo/h_hi
  (node halves) so phase 2's low-half gathers can start while the high half
  is still being computed.
- Per dst block and src half: one batched SWDGE dma_gather pulls all C*128
  source rows h[src] into SBUF in [lane, chunk, feat] layout (idx i ->
  partition i%128, chunk i//128), amortizing the ~1us fixed SWDGE cost over
  the whole block instead of paying it per 128-row chunk. int16 gather
  indices only span 32K rows, hence the lo/hi split.
- A value-scaled one-hot matrix P (DVE iota/is_equal, bf16) is matmul'd
  against the messages, accumulating into PSUM:
  psum[d, f] = sum_e val[e] * [dst_e == d] * h[src_e, f]; low-half partial
  sums park in SBUF f32; high-half pass adds, relu, store.
- Padding edges carry val = 0 and gather row 0, so they contribute nothing.
"""
import math
from contextlib import ExitStack

import numpy as np
import ml_dtypes

import concourse.bass as bass
import concourse.mybir as mybir
import concourse.tile as tile
from concourse.bass_utils import run_bass_kernel_spmd
from concourse.library_config import mlp
from concourse.library_overlay import lower_extended_insts
from concourse.vector_clock import ScopedClock

# --- workaround: this walrus build rejects >1 sync wait per instruction
# ("Too many sync wait commands"). Tile's kernel-tail drain carries one wait
# per active sem lane; give it the same NOP-splitting treatment as everything
# else via a post-schedule legalization pass over all basic blocks. ---
_MAX_WAITS = 1


def _patched_drain_and_barrier(self, tick_clock, wait_clock):
    drain_inst = self.nc.sync.drain()
    wait_clock.add_sem_waits(
        drain_inst.ins, ScopedClock({None: tick_clock.global_clock})
    )
    self.nc.all_engine_barrier()
    popped = self.nc._tile_sem_poison_stack.pop()
    assert popped is self._sem_poison
    self.nc.clear_and_free_semaphores(list(self.sems.allocated().values()))
    self.nc.all_engine_barrier()


tile.TileContext._drain_and_barrier = _patched_drain_and_barrier


def _legalize_sync_waits(nc):
    """Split instructions carrying >_MAX_WAITS sem waits: excess waits move to
    same-engine NOPs inserted immediately before the instruction."""
    n_split = 0
    for f in nc.m.functions:
        for bb in f.blocks:
            out = []
            changed = False
            for ins in bb.instructions:
                si = ins.sync_info
                waits = list(si.on_wait) if si and si.on_wait else []
                if len(waits) > _MAX_WAITS:
                    changed = True
                    n_split += 1
                    for i in range(_MAX_WAITS, len(waits), _MAX_WAITS):
                        nop = mybir.InstNoOp(
                            name=nc.get_next_instruction_name(), ins=[], outs=[]
                        )
                        nop.engine = ins.engine
                        nop.sync_info = mybir.SyncInfo(
                            on_wait=waits[i : i + _MAX_WAITS], on_update=[]
                        )
                        nc.register_instruction(nop, overwrite=True)
                        out.append(nop)
                    si.on_wait = waits[:_MAX_WAITS]
                out.append(ins)
            if changed:
                bb.instructions = out
    return n_split

N_CORES = 8
P = 128


def build_nc(n_pad, H, d_in, d_out, blocks, C_lo, C_hi, strip_blocks=8):
    """One SPMD program. n_pad: padded node count (mult of 128), H: low-half
    row count (mult of 128, < 32768). blocks: dst blocks per core. C_lo/C_hi:
    chunks (of 128 edges) per block for the low/high src halves."""
    f32 = mybir.dt.float32
    bf16 = mybir.dt.bfloat16
    i16 = mybir.dt.int16
    KD = d_in // P
    C = C_lo + C_hi
    Cmax = max(C_lo, C_hi)
    NB_lo = H // P
    NB_hi = (n_pad - H) // P

    nc = bass.Bass(num_swdge_queues=4)
    xT = nc.declare_dram_parameter("xT", [d_in, n_pad], bf16, isOutput=False)
    Wp = nc.declare_dram_parameter("W", [d_in, d_out], bf16, isOutput=False)
    idxp = nc.declare_dram_parameter("idx", [P, blocks * C * 8], i16, isOutput=False)
    dstp = nc.declare_dram_parameter("dstv", [P, blocks * C], bf16, isOutput=False)
    valp = nc.declare_dram_parameter("valv", [P, blocks * C], bf16, isOutput=False)
    outp = nc.declare_dram_parameter("out", [blocks * P, d_out], f32, isOutput=True)
    h_lo = nc.dram_tensor("h_lo", [H, d_out], bf16)
    h_hi = nc.dram_tensor("h_hi", [n_pad - H, d_out], bf16)

    with tile.TileContext(nc) as tc:
        with ExitStack() as ctx:
            wpool = ctx.enter_context(tc.tile_pool(name="w", bufs=1))
            epool = ctx.enter_context(tc.tile_pool(name="edges", bufs=1))
            xpool = ctx.enter_context(tc.tile_pool(name="xs", bufs=2))
            hpool = ctx.enter_context(tc.tile_pool(name="hs", bufs=2))
            ps1 = ctx.enter_context(tc.tile_pool(name="ps1", bufs=4, space="PSUM"))
            mpool = ctx.enter_context(tc.tile_pool(name="msgs", bufs=8))
            ppool = ctx.enter_context(tc.tile_pool(name="onehot", bufs=4))
            ps2 = ctx.enter_context(tc.tile_pool(name="ps2", bufs=4, space="PSUM"))
            lpool = ctx.enter_context(tc.tile_pool(name="outlo", bufs=1))
            opool = ctx.enter_context(tc.tile_pool(name="osb", bufs=3))

            # --- constants / per-core edge data, loaded once ---
            # iota must precede load_library(mlp): InstIota lives in the
            # default library overlay.
            iota_t = epool.tile([P, Cmax * P], bf16)
            nc.gpsimd.iota(
                iota_t[:],
                pattern=[[0, Cmax], [1, P]],
                base=0,
                channel_multiplier=0,
                allow_small_or_imprecise_dtypes=True,
            )
            nc.gpsimd.load_library(mlp)
            # Preallocate the gather-count registers now: to_reg's scratch
            # pool is exhausted once the full phase-1 program has been
            # emitted, so late allocation fails at this problem size.
            # The SWDGE ring rejects >1024 descriptors per instruction, so
            # gathers are split into runs of <= GMAX chunks.
            GMAX = 8

            def split_chunks(cn):
                n_sub = -(-cn // GMAX)
                base, rem = divmod(cn, n_sub)
                return [base + (1 if i < rem else 0) for i in range(n_sub)]

            sizes = {g for cn in (C_lo, C_hi) for g in split_chunks(cn)}
            gregs = {g: nc.gpsimd.to_reg(g * P) for g in sorted(sizes)}
            w_t = wpool.tile([P, KD * d_out], bf16)
            for k in range(KD):
                nc.sync.dma_start(
                    w_t[:, k * d_out : (k + 1) * d_out], Wp[k * P : (k + 1) * P, :]
                )
            idx_t = epool.tile([P, blocks * C * 8], i16)
            dst_t = epool.tile([P, blocks * C], bf16)
            val_t = epool.tile([P, blocks * C], bf16)
            nc.sync.dma_start(idx_t[:], idxp[:])
            nc.sync.dma_start(dst_t[:], dstp[:])
            nc.sync.dma_start(val_t[:], valp[:])

            # --- phase 1: h = x @ W (bf16, replicated), low half first ---
            SBN = strip_blocks * P  # nodes per strip
            for hdst, nb_half, col0 in ((h_lo, NB_lo, 0), (h_hi, NB_hi, H)):
                for s0 in range(0, nb_half, strip_blocks):
                    nbc = min(strip_blocks, nb_half - s0)
                    sn = nbc * P
                    xs = xpool.tile([P, KD * SBN], bf16, tag="xs")
                    for k in range(KD):
                        nc.sync.dma_start(
                            xs[:, k * SBN : k * SBN + sn],
                            xT[k * P : (k + 1) * P, col0 + s0 * P : col0 + s0 * P + sn],
                        )
                    hs = hpool.tile([P, strip_blocks * d_out], bf16, tag="hs")
                    for j in range(nbc):
                        pt = ps1.tile([P, d_out], f32)
                        for k in range(KD):
                            nc.tensor.matmul(
                                pt[:],
                                lhsT=xs[:, k * SBN + j * P : k * SBN + (j + 1) * P],
                                rhs=w_t[:, k * d_out : (k + 1) * d_out],
                                start=(k == 0),
                                stop=(k == KD - 1),
                            )
                        nc.scalar.activation(
                            hs[:, j * d_out : (j + 1) * d_out],
                            pt[:],
                            mybir.ActivationFunctionType.Copy,
                        )
                    hd = hdst[s0 * P : s0 * P + sn, :].rearrange(
                        "(nb p) f -> p nb f", p=P
                    )
                    hsv = hs[:, : nbc * d_out].rearrange("p (nb f) -> p nb f", nb=nbc)
                    nc.sync.dma_start(hd, hsv)

            # --- phase 2: batched gather + one-hot scatter matmul per block ---
            out_lo = lpool.tile([P, blocks * d_out], f32)

            qctr = [0]

            def half_pass(b, hsrc, c0, cn):
                """Gather cn chunks (cols c0..c0+cn of block b's edge arrays)
                from hsrc; one-hot scatter-matmul them into a PSUM acc."""
                msgs = mpool.tile([P, cn * d_out], bf16, tag="msgs")
                off = 0
                for g in split_chunks(cn):
                    nc.gpsimd.dma_gather(
                        msgs[:, off * d_out : (off + g) * d_out].rearrange(
                            "p (c f) -> p c f", f=d_out
                        ),
                        hsrc[:],
                        idx_t[:, (b * C + c0 + off) * 8 : (b * C + c0 + off + g) * 8],
                        g * P,
                        gregs[g],
                        d_out,
                        queue_num=qctr[0] % 4,
                    )
                    qctr[0] += 1
                    off += g
                pt3 = ppool.tile([P, cn * P], bf16, tag="P")
                iota3 = bass.AP(
                    iota_t[:].tensor, iota_t[:].offset,
                    [iota_t[:].ap[0], [P, cn], [1, P]],
                )
                p3 = bass.AP(
                    pt3[:].tensor, pt3[:].offset, [pt3[:].ap[0], [P, cn], [1, P]]
                )
                dstb = dst_t[:, b * C + c0 : b * C + c0 + cn]
                valb = val_t[:, b * C + c0 : b * C + c0 + cn]
                dst_bc = bass.AP(dstb.tensor, dstb.offset, dstb.ap + [[0, P]])
                val_bc = bass.AP(valb.tensor, valb.offset, valb.ap + [[0, P]])
                nc.vector.tensor_tensor(
                    out=p3, in0=iota3, in1=dst_bc, op=mybir.AluOpType.is_equal
                )
                nc.vector.tensor_tensor(
                    out=p3, in0=p3, in1=val_bc, op=mybir.AluOpType.mult
                )
                acc = ps2.tile([P, d_out], f32)
                for c in range(cn):
                    nc.tensor.matmul(
                        acc[:],
                        lhsT=pt3[:, c * P : (c + 1) * P],
                        rhs=msgs[:, c * d_out : (c + 1) * d_out],
                        start=(c == 0),
                        stop=(c == cn - 1),
                    )
                return acc

            for b in range(blocks):  # pass A: low-half src
                acc = half_pass(b, h_lo, 0, C_lo)
                nc.vector.tensor_copy(out_lo[:, b * d_out : (b + 1) * d_out], acc[:])
            for b in range(blocks):  # pass B: high-half src + combine
                acc = half_pass(b, h_hi, C_lo, C_hi)
                ot = opool.tile([P, d_out], f32)
                nc.vector.tensor_tensor(
                    out=ot[:],
                    in0=acc[:],
                    in1=out_lo[:, b * d_out : (b + 1) * d_out],
                    op=mybir.AluOpType.add,
                )
                ot2 = opool.tile([P, d_out], f32)
                nc.scalar.activation(ot2[:], ot[:], mybir.ActivationFunctionType.Relu)
                nc.sync.dma_start(outp[b * P : (b + 1) * P, :], ot2[:])
    lower_extended_insts(nc)
    _legalize_sync_waits(nc)
    return nc


def _pack_edges(edge_src, edge_dst, edge_vals, n_nodes, blocks, H):
    """Permute dst nodes into degree-balanced blocks of 128; split each
    block's edges by src half (< H vs >= H); pack into per-core arrays:
    int16 gather indices (16-partition-wrapped, replicated to 128) plus
    dst-slot/val arrays in [lane, block*C+chunk] layout."""
    import heapq

    total_blocks = N_CORES * blocks
    deg = np.bincount(edge_dst, minlength=n_nodes).astype(np.int64)
    order = np.argsort(-deg, kind="stable")
    heap = [(0, b) for b in range(total_blocks)]
    heapq.heapify(heap)
    counts = np.zeros(total_blocks, np.int32)
    loads = np.zeros(total_blocks, np.int64)
    block_of = np.empty(n_nodes, np.int32)
    slot_of = np.empty(n_nodes, np.int32)
    for node in order:
        while True:
            load, b = heapq.heappop(heap)
            if counts[b] < P:
                break
        block_of[node] = b
        slot_of[node] = counts[b]
        counts[b] += 1
        loads[b] = load + deg[node]
        if counts[b] < P:
            heapq.heappush(heap, (loads[b], b))

    E = len(edge_src)
    eb = block_of[edge_dst].astype(np.int64)
    ishi = (edge_src >= H).astype(np.int64)
    g = eb * 2 + ishi
    eorder = np.argsort(g, kind="stable")
    gs = g[eorder]
    gcounts = np.bincount(gs, minlength=total_blocks * 2)
    C_lo = max(1, math.ceil(gcounts[0::2].max() / P))
    C_hi = max(1, math.ceil(gcounts[1::2].max() / P))
    C = C_lo + C_hi
    goffs = np.concatenate([[0], np.cumsum(gcounts)[:-1]])
    pos = np.arange(E) - goffs[gs]
    ch = np.where(gs % 2 == 0, 0, C_lo) + (pos >> 7)
    lane = pos & 127
    blk = gs >> 1

    idx_a = np.zeros((total_blocks, C, P), np.int16)
    dst_a = np.zeros((total_blocks, C, P), np.float32)
    val_a = np.zeros((total_blocks, C, P), np.float32)
    src_o = edge_src[eorder]
    idx_a[blk, ch, lane] = np.where(src_o < H, src_o, src_o - H).astype(np.int16)
    dst_a[blk, ch, lane] = slot_of[edge_dst[eorder]]
    val_a[blk, ch, lane] = edge_vals[eorder]

    # 16-partition wrap per (block, half): idx j -> (j%16, j//16), chunks
    # flattened chunk-major; lo cols then hi cols; replicate to 128 parts.
    lo_w = idx_a[:, :C_lo].reshape(total_blocks, -1, 16).transpose(0, 2, 1)
    hi_w = idx_a[:, C_lo:].reshape(total_blocks, -1, 16).transpose(0, 2, 1)
    idx_w = np.concatenate([lo_w, hi_w], axis=2)  # [tb, 16, C*8]
    bf = ml_dtypes.bfloat16

    idx_c, dst_c, val_c = [], [], []
    for m in range(N_CORES):
        iw = idx_w[m * blocks : (m + 1) * blocks]  # [blocks, 16, C*8]
        iw = iw.transpose(1, 0, 2).reshape(16, -1)  # [16, blocks*C*8]
        idx_c.append(np.ascontiguousarray(np.tile(iw, (8, 1))))
        for a, dst in ((dst_a, dst_c), (val_a, val_c)):
            t = a[m * blocks : (m + 1) * blocks].transpose(2, 0, 1).reshape(P, -1)
            dst.append(np.ascontiguousarray(t.astype(bf)))
    return idx_c, dst_c, val_c, block_of, slot_of, C_lo, C_hi


def _run(x, W, edge_vals, edge_src, edge_dst, blocks=None, trace=False):
    n_nodes, d_in = x.shape
    d_out = W.shape[1]
    if blocks is None:
        blocks = math.ceil(n_nodes / (N_CORES * P))
    n_pad = math.ceil(n_nodes / P) * P
    H = (math.ceil(n_pad / P / 2)) * P  # low-half rows; < 32768 for int16
    assert H < 32768 and (n_pad - H) < 32768
    idx_c, dst_c, val_c, block_of, slot_of, C_lo, C_hi = _pack_edges(
        edge_src, edge_dst, edge_vals, n_nodes, blocks, H
    )
    bf = ml_dtypes.bfloat16
    xp = np.zeros((n_pad, d_in), np.float32)
    xp[:n_nodes] = x
    xT = np.ascontiguousarray(xp.T.astype(bf))
    Wb = np.ascontiguousarray(W.astype(bf))

    nc = build_nc(n_pad, H, d_in, d_out, blocks, C_lo, C_hi)
    in_maps = [
        {"xT": xT, "W": Wb, "idx": idx_c[m], "dstv": dst_c[m], "valv": val_c[m]}
        for m in range(N_CORES)
    ]
    res = run_bass_kernel_spmd(nc, in_maps, list(range(N_CORES)), trace=trace)
    stacked = np.concatenate([res.results[m]["out"] for m in range(N_CORES)], axis=0)
    npc = blocks * P
    gidx = (block_of // blocks) * npc + (block_of % blocks) * P + slot_of
    out = stacked[gidx]
    return out, res


def kernel(x, W, edge_vals, edge_src, edge_dst):
    x = np.asarray(x, np.float32)
    W = np.asarray(W, np.float32)
    edge_vals = np.asarray(edge_vals, np.float32)
    edge_src = np.asarray(edge_src).astype(np.int64)
    edge_dst = np.asarray(edge_dst).astype(np.int64)
    out, _ = _run(x, W, edge_vals, edge_src, edge_dst)
    return out.astype(np.float32)
